# revision 23
# baseline (speedup 1.0000x reference)
"""Trainium2 Bass kernel for nn_CombinatorialClassifier (segment_reduce).

Strategy (8 NeuronCores, tensor-parallel over the num_partitionings axis):
  - Core i owns partitionings {2i, 2i+1} (= A, B): a [2000, 2048] slice
    of W.
  - On device, per partitioning h in (A, B): logits_h = x @ W_h.T in
    fp8-e4m3 (+ fp16 b row folded into the matmul; quantization noise
    on the logits is ~0.05 abs, far inside the 2e-2 rel-err budget of
    the final log-probs), softmax -> probs_h [64, 1000] fp16,
    repacked into a batch-interleaved gather table
    TBL_h[ch, m, j] = probs_h[(ch%16) + 16*j, m] (d=4) replicated
    across all 8 Q7 core groups.  The A pipeline runs first so A
    gathers start while B is still in matmul/softmax.
  - Gather: one ap_gather index fetches the probs for FOUR batch rows
    at once, and each Q7 core group owns its own 6250-class range, so
    each Q7 core processes 2*6250 indices instead of 50000 -- ~3.3x
    less gpsimd command overhead than the d=1 layout.  Gather results
    are DMAed straight to a DRAM scratch in gather layout; the out-DMAs
    are issued from the scalar engine's queue so they cannot
    head-of-line-block the sync engine's serial DMA issue stream (which
    the B-table build needs), and no vector-engine work runs during the
    gathers (gpsimd shares its SBUF port with the vector engine).
  - Host: sum the 16 partials (8 cores x 2 partitionings) in the
    scratch layout, undo the batch interleave with one transpose, then
    normalize over classes and log.
"""

import os
from contextlib import ExitStack

import numpy as np

import concourse.bacc as bacc
import concourse.mybir as mybir
import concourse.tile as tile
from concourse import bass_utils
from concourse import library_config

B, P, K, C, D = 64, 16, 1000, 50000, 2048
ESP = 1e-20
NCORES = 8
PPC = P // NCORES        # partitionings per core (2)
NLOC = PPC * K           # local logits width (2000)
NT = 500                 # matmul N-tile (PSUM bank: 500 fp32 <= 512)
DCH = D // 128           # 16 contraction chunks of 128
WCH = 4                  # contraction chunks per W DMA

NG = 8                   # Q7 core groups (16 partitions each)
CG = C // NG             # classes per group (6250)
NCH = 5                  # gather chunks per group per partitioning
JCC = CG // NCH          # classes per chunk (1250)
NIDX = ((JCC + 3) // 4) * 4          # padded gather indices per call (1252)
ICOLS = ((NIDX + 31) // 32) * 2      # int16 idx cols per call (80)
ICT = 2 * NCH * ICOLS                # total idx cols (800)
SCR_W = 2 * NCH * NIDX * 4           # scratch free width (50080)
KP = 1024                            # table entries incl. PSUM pad holes

_F32 = mybir.dt.float32
_F8 = mybir.dt.float8e4
_F16 = mybir.dt.float16
_I16 = mybir.dt.int16

_CACHE = {}
LAST_RESULTS = None


def _build_nc():
    nc = bacc.Bacc(
        "TRN2",
        target_bir_lowering=False,
        debug=False,
        enable_asserts=False,
        num_devices=NCORES,
    )
    xT_d = nc.dram_tensor("xT", [D, B], _F8, kind="ExternalInput")
    w8_d = nc.dram_tensor("w8", [D, NLOC], _F8, kind="ExternalInput")
    bv_d = nc.dram_tensor("bv", [1, NLOC], _F16, kind="ExternalInput")
    idx_d = nc.dram_tensor("idx", [128, ICT], _I16, kind="ExternalInput")
    out_d = nc.dram_tensor("part_out", [128, SCR_W], _F8, kind="ExternalOutput")

    with tile.TileContext(nc) as tc, ExitStack() as ctx:
        const = ctx.enter_context(tc.tile_pool(name="const", bufs=1))
        wpool = ctx.enter_context(tc.tile_pool(name="w", bufs=3))
        spool = ctx.enter_context(tc.tile_pool(name="stats", bufs=1))
        gpool = ctx.enter_context(tc.tile_pool(name="g", bufs=3))
        psum = ctx.enter_context(
            tc.tile_pool(name="psum", bufs=1, space="PSUM")
        )

        xt = const.tile([128, DCH, B], _F8)
        nc.sync.dma_start(xt[:], xT_d.ap().rearrange("(c p) b -> p c b", p=128))
        ones = const.tile([1, B], _F16)
        nc.vector.memset(ones[:], 1.0)
        bias = const.tile([1, NLOC], _F16)
        nc.sync.dma_start(bias[:], bv_d.ap())
        idx_sb = const.tile([128, ICT], _I16)
        nc.sync.dma_start(idx_sb[:], idx_d.ap())

        # get the gpsimd library load off the critical path: it can load
        # during the matmul instead of right before the first gather
        nc.gpsimd.load_library(library_config.ap_gather)

        # logits PSUM tiles for both partitionings; the 12 pad columns of
        # each 512-wide bank are preset to -1e30 so the softmax can run as
        # single flattened [B, 1024] ops (max unaffected, exp(pad) -> 0)
        pss = []
        for h in range(PPC):
            ps = psum.tile([B, 2, 512], _F32, tag=f"ps{h}", name=f"ps{h}")
            nc.vector.memset(ps[:, :, NT:512], -1e30)
            pss.append(ps)

        TBLs = []
        # ---- per-partitioning pipeline: matmul -> softmax -> table ----
        # A (h=0) runs fully first so its gathers can start while B is
        # still in flight on the tensor/vector/scalar engines.
        for h in range(PPC):
            k0 = K * h
            ps = pss[h]
            wt = None
            for j in range(DCH):
                if j % WCH == 0:
                    wt = wpool.tile([128, WCH, K], _F8, tag="w")
                    nc.sync.dma_start(
                        wt[:],
                        w8_d[128 * j : 128 * (j + WCH), k0 : k0 + K].rearrange(
                            "(c p) n -> p c n", p=128
                        ),
                    )
                for n in range(2):
                    nc.tensor.matmul(
                        ps[:, n, 0:NT],
                        xt[:, j, :],
                        wt[:, j % WCH, NT * n : NT * (n + 1)],
                        start=(j == 0),
                        stop=False,
                    )
            for n in range(2):
                nc.tensor.matmul(
                    ps[:, n, 0:NT],
                    ones[:],
                    bias[:, k0 + NT * n : k0 + NT * (n + 1)],
                    start=False,
                    stop=True,
                )

            # softmax as single flattened [B, 1024] ops (pads hold -1e30)
            psf = ps[:, :, :].rearrange("p a b -> p (a b)")
            neg = spool.tile([B, 1], _F32, tag=f"neg{h}")
            nc.vector.reduce_max(neg[:], psf, axis=mybir.AxisListType.X)
            nc.vector.tensor_scalar_mul(neg[:], neg[:], -1.0)
            exp32 = spool.tile([B, KP], _F32, tag=f"exp{h}")
            sacc = spool.tile([B, 1], _F32, tag=f"sacc{h}")
            nc.scalar.activation(
                exp32[:],
                psf,
                mybir.ActivationFunctionType.Exp,
                bias=neg[:],
                accum_out=sacc[:],
            )
            rec = spool.tile([B, 1], _F32, tag=f"rec{h}")
            nc.vector.reciprocal(rec[:], sacc[:])
            # scale probs x512 into fp8-e4m3's normal range (probs ~1e-3
            # sit at the subnormal floor otherwise); the uniform scale
            # cancels in the final class-normalization on the host
            nc.vector.tensor_scalar_mul(rec[:], rec[:], 512.0)
            probs16 = spool.tile([B, KP], _F16, tag=f"p16{h}")
            nc.vector.tensor_scalar_mul(probs16[:], exp32[:], rec[:])

            # batch-interleaved gather table:
            # TBL[ch, m, j] = probs16[(ch % 16) + 16*j, m].
            # stage[ch, j, m] first: 4 interleave DMAs into group 0, then
            # 7 independent one-level replication DMAs (all contiguous;
            # a strided DMA would lower to a catastrophically slow
            # 2-byte-element transfer).  One strided vector copy then
            # does the (j, m) -> (m, j) interleave.
            stage = spool.tile([128, 4, KP], _F16, tag=f"st{h}")
            for j in range(4):
                nc.sync.dma_start(
                    stage[0:16, j : j + 1, :],
                    probs16[16 * j : 16 * (j + 1), :].unsqueeze(1),
                )
            for q in range(1, NG):
                nc.sync.dma_start(
                    stage[16 * q : 16 * (q + 1), :, :], stage[0:16, :, :]
                )
            TBL = spool.tile([128, KP, 4], _F8, tag=f"tbl{h}")
            nc.vector.tensor_copy(
                TBL[:, :, :].transpose([0, 2, 1]), stage[:, :, :]
            )
            TBLs.append(TBL)

        # ---- gather + store (gather layout; host sums and reorders) ----
        # chunk t: Q7 group q handles classes [CG*q + JCC*t, CG*q + JCC*(t+1))
        for t in range(NCH):
            for h in range(PPC):
                g = gpool.tile([128, NIDX, 4], _F8, tag="g")
                nc.gpsimd.ap_gather(
                    g[:, :, :],
                    TBLs[h][:, :, :],
                    idx_sb[:, ICOLS * (NCH * h + t) : ICOLS * (NCH * h + t + 1)],
                    channels=128,
                    num_elems=KP,
                    d=4,
                    num_idxs=NIDX,
                )
                s = 2 * t + h
                nc.scalar.dma_start(
                    out_d.ap().rearrange(
                        "p (s a b) -> p s a b", s=2 * NCH, a=NIDX
                    )[:, s, :, :],
                    g[:, :, :],
                )

    nc.compile()
    return nc


def _host_inputs(x, W, b, part):
    """Per-core input maps: xT, wtb (W.T shard + bias row), gather indices."""
    import ml_dtypes

    xT = np.ascontiguousarray(x.T.astype(ml_dtypes.float8_e4m3))
    part = np.asarray(part).astype(np.int64, copy=False)
    in_maps = []
    for i in range(NCORES):
        r0 = NLOC * i
        w8 = np.ascontiguousarray(
            W[r0 : r0 + NLOC].T.astype(ml_dtypes.float8_e4m3)
        )
        bv = b[r0 : r0 + NLOC].astype(np.float16).reshape(1, NLOC)

        idxh = np.zeros((128, ICT), np.int16)
        for h in range(PPC):
            ph = (part[2 * i + h] - (2 * i + h) * K).astype(np.int16)  # [C] in [0, K)
            ph = ph + 12 * (ph >= 500)  # skip the PSUM pad holes in TBL
            for q in range(NG):
                for t in range(NCH):
                    c0 = CG * q + JCC * t
                    s = np.zeros(ICOLS * 16, np.int16)
                    s[0:JCC] = ph[c0 : c0 + JCC]
                    blk = s.reshape(ICOLS, 16).T  # pos i <- (partition i%16, col i//16)
                    col = ICOLS * (NCH * h + t)
                    idxh[16 * q : 16 * (q + 1), col : col + ICOLS] = blk
        in_maps.append({"xT": xT, "w8": w8, "bv": bv, "idx": idxh})
    return in_maps


def kernel(**inputs):
    global LAST_RESULTS
    x = np.asarray(inputs["input"], dtype=np.float32)
    W = np.asarray(inputs["W"], dtype=np.float32)
    b = np.asarray(inputs["b"], dtype=np.float32)
    part = np.asarray(inputs["partitionings"])
    assert x.shape == (B, D) and W.shape == (P * K, D)

    if "nc" not in _CACHE:
        _CACHE["nc"] = _build_nc()
    nc = _CACHE["nc"]

    in_maps = _host_inputs(x, W, b, part)
    trace = bool(int(os.environ.get("BASSK_TRACE", "0")))
    res = bass_utils.run_bass_kernel_spmd(
        nc,
        in_maps,
        core_ids=list(range(NCORES)),
        trace=trace,
        tmpdir=os.environ.get("BASSK_TRACE_DIR") or None,
    )
    LAST_RESULTS = res

    # scratch layout: [128, t(NCH), h(PPC), NIDX, 4];
    # value at (ch, t, h, cc, j) = probs_{2*core+h}[b, part(...)] for
    # b = (ch%16) + 16*j, class c = CG*(ch//16) + JCC*t + cc (cc < JCC).
    acc = np.zeros((128, NCH, NIDX, 4), np.float32)
    for i in range(NCORES):
        arr = res.results[i]["part_out"].reshape(128, NCH, PPC, NIDX, 4)
        acc += arr[:, :, 0].astype(np.float32)
        acc += arr[:, :, 1].astype(np.float32)
    acc = acc[:, :, :JCC, :]                       # drop idx padding
    # (q, p', t, cc, j) -> out[b = p' + 16j, c = (q, t, cc)]
    out = np.ascontiguousarray(
        acc.reshape(NG, 16, NCH, JCC, 4).transpose(4, 1, 0, 2, 3)
    ).reshape(B, C)
    tot = out.sum(axis=1, keepdims=True)
    return np.log(out / tot + ESP).astype(np.float32)


# revision 24
# speedup vs baseline: 1.0113x; 1.0113x over previous
"""Trainium2 Bass kernel for nn_CombinatorialClassifier (segment_reduce).

Strategy (8 NeuronCores, tensor-parallel over the num_partitionings axis):
  - Core i owns partitionings {2i, 2i+1} (= A, B): a [2000, 2048] slice
    of W.
  - On device, per partitioning h in (A, B): logits_h = x @ W_h.T in
    fp8-e4m3 (+ fp16 b row folded into the matmul; quantization noise
    on the logits is ~0.05 abs, far inside the 2e-2 rel-err budget of
    the final log-probs), softmax -> probs_h [64, 1000] fp16,
    repacked into a batch-interleaved gather table
    TBL_h[ch, m, j] = probs_h[(ch%16) + 16*j, m] (d=4) replicated
    across all 8 Q7 core groups.  The A pipeline runs first so A
    gathers start while B is still in matmul/softmax.
  - Gather: one ap_gather index fetches the probs for FOUR batch rows
    at once, and each Q7 core group owns its own 6250-class range, so
    each Q7 core processes 2*6250 indices instead of 50000 -- ~3.3x
    less gpsimd command overhead than the d=1 layout.  Gather results
    are DMAed straight to a DRAM scratch in gather layout; the out-DMAs
    are issued from the scalar engine's queue so they cannot
    head-of-line-block the sync engine's serial DMA issue stream (which
    the B-table build needs), and no vector-engine work runs during the
    gathers (gpsimd shares its SBUF port with the vector engine).
  - Host: sum the 16 partials (8 cores x 2 partitionings) in the
    scratch layout, undo the batch interleave with one transpose, then
    normalize over classes and log.
"""

import os
from contextlib import ExitStack

import numpy as np

import concourse.bacc as bacc
import concourse.mybir as mybir
import concourse.tile as tile
from concourse import bass_utils
from concourse import library_config

B, P, K, C, D = 64, 16, 1000, 50000, 2048
ESP = 1e-20
NCORES = 8
PPC = P // NCORES        # partitionings per core (2)
NLOC = PPC * K           # local logits width (2000)
NT = 500                 # matmul N-tile (PSUM bank: 500 fp32 <= 512)
DCH = D // 128           # 16 contraction chunks of 128
WCH = 4                  # contraction chunks per W DMA

NG = 8                   # Q7 core groups (16 partitions each)
CG = C // NG             # classes per group (6250)
NCH = 5                  # gather chunks per group per partitioning
JCC = CG // NCH          # classes per chunk (1250)
NIDX = ((JCC + 3) // 4) * 4          # padded gather indices per call (1252)
ICOLS = ((NIDX + 31) // 32) * 2      # int16 idx cols per call (80)
ICT = 2 * NCH * ICOLS                # total idx cols (800)
SCR_W = 2 * NCH * NIDX * 4           # scratch free width (50080)

_F32 = mybir.dt.float32
_F8 = mybir.dt.float8e4
_F16 = mybir.dt.float16
_I16 = mybir.dt.int16

_CACHE = {}
LAST_RESULTS = None


def _build_nc():
    nc = bacc.Bacc(
        "TRN2",
        target_bir_lowering=False,
        debug=False,
        enable_asserts=False,
        num_devices=NCORES,
    )
    xT_d = nc.dram_tensor("xT", [D, B], _F8, kind="ExternalInput")
    w8_d = nc.dram_tensor("w8", [D, NLOC], _F8, kind="ExternalInput")
    bv_d = nc.dram_tensor("bv", [1, NLOC], _F16, kind="ExternalInput")
    idx_d = nc.dram_tensor("idx", [128, ICT], _I16, kind="ExternalInput")
    out_d = nc.dram_tensor("part_out", [128, SCR_W], _F8, kind="ExternalOutput")

    with tile.TileContext(nc) as tc, ExitStack() as ctx:
        const = ctx.enter_context(tc.tile_pool(name="const", bufs=1))
        wpool = ctx.enter_context(tc.tile_pool(name="w", bufs=3))
        spool = ctx.enter_context(tc.tile_pool(name="stats", bufs=1))
        gpool = ctx.enter_context(tc.tile_pool(name="g", bufs=3))
        psum = ctx.enter_context(
            tc.tile_pool(name="psum", bufs=1, space="PSUM")
        )

        xt = const.tile([128, DCH, B], _F8)
        nc.sync.dma_start(xt[:], xT_d.ap().rearrange("(c p) b -> p c b", p=128))
        ones = const.tile([1, B], _F16)
        nc.vector.memset(ones[:], 1.0)
        bias = const.tile([1, NLOC], _F16)
        nc.sync.dma_start(bias[:], bv_d.ap())
        idx_sb = const.tile([128, ICT], _I16)
        nc.sync.dma_start(idx_sb[:], idx_d.ap())

        # get the gpsimd library load off the critical path: it can load
        # during the matmul instead of right before the first gather
        nc.gpsimd.load_library(library_config.ap_gather)

        TBLs = []
        # ---- per-partitioning pipeline: matmul -> softmax -> table ----
        # A (h=0) runs fully first so its gathers can start while B is
        # still in flight on the tensor/vector/scalar engines.
        for h in range(PPC):
            k0 = K * h
            ps = psum.tile([B, 2, 512], _F32, tag=f"ps{h}", name=f"ps{h}")
            wt = None
            for j in range(DCH):
                if j % WCH == 0:
                    wt = wpool.tile([128, WCH, K], _F8, tag="w")
                    nc.sync.dma_start(
                        wt[:],
                        w8_d[128 * j : 128 * (j + WCH), k0 : k0 + K].rearrange(
                            "(c p) n -> p c n", p=128
                        ),
                    )
                for n in range(2):
                    nc.tensor.matmul(
                        ps[:, n, 0:NT],
                        xt[:, j, :],
                        wt[:, j % WCH, NT * n : NT * (n + 1)],
                        start=(j == 0),
                        stop=False,
                    )
            for n in range(2):
                nc.tensor.matmul(
                    ps[:, n, 0:NT],
                    ones[:],
                    bias[:, k0 + NT * n : k0 + NT * (n + 1)],
                    start=False,
                    stop=True,
                )

            # softmax over the two PSUM banks of this partitioning
            mx = spool.tile([B, 2], _F32, tag=f"mx{h}")
            for n in range(2):
                nc.vector.reduce_max(
                    mx[:, n : n + 1], ps[:, n, 0:NT], axis=mybir.AxisListType.X
                )
            neg = spool.tile([B, 1], _F32, tag=f"neg{h}")
            nc.vector.tensor_tensor(
                neg[:], mx[:, 0:1], mx[:, 1:2], op=mybir.AluOpType.max
            )
            nc.vector.tensor_scalar_mul(neg[:], neg[:], -1.0)
            exp32 = spool.tile([B, K], _F32, tag=f"exp{h}")
            sacc = spool.tile([B, 2], _F32, tag=f"sacc{h}")
            for n in range(2):
                nc.scalar.activation(
                    exp32[:, NT * n : NT * (n + 1)],
                    ps[:, n, 0:NT],
                    mybir.ActivationFunctionType.Exp,
                    bias=neg[:],
                    accum_out=sacc[:, n : n + 1],
                )
            rec = spool.tile([B, 1], _F32, tag=f"rec{h}")
            nc.vector.tensor_tensor(
                rec[:], sacc[:, 0:1], sacc[:, 1:2], op=mybir.AluOpType.add
            )
            nc.vector.reciprocal(rec[:], rec[:])
            # scale probs x512 into fp8-e4m3's normal range (probs ~1e-3
            # sit at the subnormal floor otherwise); the uniform scale
            # cancels in the final class-normalization on the host
            nc.vector.tensor_scalar_mul(rec[:], rec[:], 512.0)
            probs16 = spool.tile([B, K], _F16, tag=f"p16{h}")
            nc.vector.tensor_scalar_mul(probs16[:], exp32[:], rec[:])

            # batch-interleaved gather table:
            # TBL[ch, m, j] = probs16[(ch % 16) + 16*j, m].
            # stage[ch, j, m] first: 4 interleave DMAs into group 0, then
            # 7 independent one-level replication DMAs (all contiguous;
            # a strided DMA would lower to a catastrophically slow
            # 2-byte-element transfer).  One strided vector copy then
            # does the (j, m) -> (m, j) interleave.
            stage = spool.tile([128, 4, K], _F16, tag=f"st{h}")
            for j in range(4):
                nc.sync.dma_start(
                    stage[0:16, j : j + 1, :],
                    probs16[16 * j : 16 * (j + 1), :].unsqueeze(1),
                )
            for q in range(1, NG):
                nc.sync.dma_start(
                    stage[16 * q : 16 * (q + 1), :, :], stage[0:16, :, :]
                )
            TBL = spool.tile([128, K, 4], _F8, tag=f"tbl{h}")
            nc.vector.tensor_copy(
                TBL[:, :, :].transpose([0, 2, 1]), stage[:, :, :]
            )
            TBLs.append(TBL)

        # ---- gather + store (gather layout; host sums and reorders) ----
        # chunk t: Q7 group q handles classes [CG*q + JCC*t, CG*q + JCC*(t+1))
        for t in range(NCH):
            for h in range(PPC):
                g = gpool.tile([128, NIDX, 4], _F8, tag="g")
                nc.gpsimd.ap_gather(
                    g[:, :, :],
                    TBLs[h][:, :, :],
                    idx_sb[:, ICOLS * (NCH * h + t) : ICOLS * (NCH * h + t + 1)],
                    channels=128,
                    num_elems=K,
                    d=4,
                    num_idxs=NIDX,
                )
                s = 2 * t + h
                nc.scalar.dma_start(
                    out_d.ap().rearrange(
                        "p (s a b) -> p s a b", s=2 * NCH, a=NIDX
                    )[:, s, :, :],
                    g[:, :, :],
                )

    nc.compile()
    return nc


def _host_inputs(x, W, b, part):
    """Per-core input maps: xT, wtb (W.T shard + bias row), gather indices."""
    import ml_dtypes

    xT = np.ascontiguousarray(x.T.astype(ml_dtypes.float8_e4m3))
    part = np.asarray(part).astype(np.int64, copy=False)
    in_maps = []
    for i in range(NCORES):
        r0 = NLOC * i
        w8 = np.ascontiguousarray(
            W[r0 : r0 + NLOC].T.astype(ml_dtypes.float8_e4m3)
        )
        bv = b[r0 : r0 + NLOC].astype(np.float16).reshape(1, NLOC)

        idxh = np.zeros((128, ICT), np.int16)
        for h in range(PPC):
            ph = (part[2 * i + h] - (2 * i + h) * K).astype(np.int16)  # [C] in [0, K)
            for q in range(NG):
                for t in range(NCH):
                    c0 = CG * q + JCC * t
                    s = np.zeros(ICOLS * 16, np.int16)
                    s[0:JCC] = ph[c0 : c0 + JCC]
                    blk = s.reshape(ICOLS, 16).T  # pos i <- (partition i%16, col i//16)
                    col = ICOLS * (NCH * h + t)
                    idxh[16 * q : 16 * (q + 1), col : col + ICOLS] = blk
        in_maps.append({"xT": xT, "w8": w8, "bv": bv, "idx": idxh})
    return in_maps


def kernel(**inputs):
    global LAST_RESULTS
    x = np.asarray(inputs["input"], dtype=np.float32)
    W = np.asarray(inputs["W"], dtype=np.float32)
    b = np.asarray(inputs["b"], dtype=np.float32)
    part = np.asarray(inputs["partitionings"])
    assert x.shape == (B, D) and W.shape == (P * K, D)

    if "nc" not in _CACHE:
        _CACHE["nc"] = _build_nc()
    nc = _CACHE["nc"]

    in_maps = _host_inputs(x, W, b, part)
    trace = bool(int(os.environ.get("BASSK_TRACE", "0")))
    res = bass_utils.run_bass_kernel_spmd(
        nc,
        in_maps,
        core_ids=list(range(NCORES)),
        trace=trace,
        tmpdir=os.environ.get("BASSK_TRACE_DIR") or None,
    )
    LAST_RESULTS = res

    # scratch layout: [128, t(NCH), h(PPC), NIDX, 4];
    # value at (ch, t, h, cc, j) = probs_{2*core+h}[b, part(...)] for
    # b = (ch%16) + 16*j, class c = CG*(ch//16) + JCC*t + cc (cc < JCC).
    acc = np.zeros((128, NCH, NIDX, 4), np.float32)
    for i in range(NCORES):
        arr = res.results[i]["part_out"].reshape(128, NCH, PPC, NIDX, 4)
        acc += arr[:, :, 0].astype(np.float32)
        acc += arr[:, :, 1].astype(np.float32)
    acc = acc[:, :, :JCC, :]                       # drop idx padding
    # (q, p', t, cc, j) -> out[b = p' + 16j, c = (q, t, cc)]
    out = np.ascontiguousarray(
        acc.reshape(NG, 16, NCH, JCC, 4).transpose(4, 1, 0, 2, 3)
    ).reshape(B, C)
    tot = out.sum(axis=1, keepdims=True)
    return np.log(out / tot + ESP).astype(np.float32)


# revision 25
# speedup vs baseline: 1.0121x; 1.0008x over previous
"""Trainium2 Bass kernel for nn_CombinatorialClassifier (segment_reduce).

Strategy (8 NeuronCores, tensor-parallel over the num_partitionings axis):
  - Core i owns partitionings {2i, 2i+1} (= A, B): a [2000, 2048] slice
    of W.
  - On device, per partitioning h in (A, B): logits_h = x @ W_h.T in
    fp8-e4m3 (+ fp16 b row folded into the matmul; quantization noise
    on the logits is ~0.05 abs, far inside the 2e-2 rel-err budget of
    the final log-probs), softmax -> probs_h [64, 1000] fp16,
    repacked into a batch-interleaved gather table
    TBL_h[ch, m, j] = probs_h[(ch%16) + 16*j, m] (d=4) replicated
    across all 8 Q7 core groups.  The A pipeline runs first so A
    gathers start while B is still in matmul/softmax.
  - Gather: one ap_gather index fetches the probs for FOUR batch rows
    at once, and each Q7 core group owns its own 6250-class range, so
    each Q7 core processes 2*6250 indices instead of 50000 -- ~3.3x
    less gpsimd command overhead than the d=1 layout.  Gather results
    are DMAed straight to a DRAM scratch in gather layout; the out-DMAs
    are issued from the scalar engine's queue so they cannot
    head-of-line-block the sync engine's serial DMA issue stream (which
    the B-table build needs), and no vector-engine work runs during the
    gathers (gpsimd shares its SBUF port with the vector engine).
  - Host: sum the 16 partials (8 cores x 2 partitionings) in the
    scratch layout, undo the batch interleave with one transpose, then
    normalize over classes and log.
"""

import os
from contextlib import ExitStack

import numpy as np

import concourse.bacc as bacc
import concourse.mybir as mybir
import concourse.tile as tile
from concourse import bass_utils
from concourse import library_config

B, P, K, C, D = 64, 16, 1000, 50000, 2048
ESP = 1e-20
NCORES = 8
PPC = P // NCORES        # partitionings per core (2)
NLOC = PPC * K           # local logits width (2000)
NT = 500                 # matmul N-tile (PSUM bank: 500 fp32 <= 512)
DCH = D // 128           # 16 contraction chunks of 128
WCH = 4                  # contraction chunks per W DMA

NG = 8                   # Q7 core groups (16 partitions each)
CG = C // NG             # classes per group (6250)
NCH = 5                  # gather chunks per group per partitioning
JCC = CG // NCH          # classes per chunk (1250)
NIDX = ((JCC + 3) // 4) * 4          # padded gather indices per call (1252)
ICOLS = ((NIDX + 31) // 32) * 2      # int16 idx cols per call (80)
ICT = 2 * NCH * ICOLS                # total idx cols (800)
SCR_W = 2 * NCH * NIDX * 4           # scratch free width (50080)

_F32 = mybir.dt.float32
_F8 = mybir.dt.float8e4
_F16 = mybir.dt.float16
_I16 = mybir.dt.int16

_CACHE = {}
LAST_RESULTS = None


def _build_nc():
    nc = bacc.Bacc(
        "TRN2",
        target_bir_lowering=False,
        debug=False,
        enable_asserts=False,
        num_devices=NCORES,
    )
    xT_d = nc.dram_tensor("xT", [128, DCH * B], _F8, kind="ExternalInput")
    w8_d = nc.dram_tensor("w8", [128, PPC * DCH * K], _F8, kind="ExternalInput")
    bv_d = nc.dram_tensor("bv", [1, NLOC], _F16, kind="ExternalInput")
    idx_d = nc.dram_tensor("idx", [128, ICT], _I16, kind="ExternalInput")
    out_d = nc.dram_tensor("part_out", [128, SCR_W], _F8, kind="ExternalOutput")

    with tile.TileContext(nc) as tc, ExitStack() as ctx:
        const = ctx.enter_context(tc.tile_pool(name="const", bufs=1))
        wpool = ctx.enter_context(tc.tile_pool(name="w", bufs=3))
        spool = ctx.enter_context(tc.tile_pool(name="stats", bufs=1))
        gpool = ctx.enter_context(tc.tile_pool(name="g", bufs=3))
        psum = ctx.enter_context(
            tc.tile_pool(name="psum", bufs=1, space="PSUM")
        )

        # host pre-permutes x and W so these DMAs are contiguous runs
        xt = const.tile([128, DCH, B], _F8)
        nc.sync.dma_start(xt[:], xT_d.ap().rearrange("p (c b) -> p c b", b=B))
        ones = const.tile([1, B], _F16)
        nc.vector.memset(ones[:], 1.0)
        bias = const.tile([1, NLOC], _F16)
        nc.sync.dma_start(bias[:], bv_d.ap())
        idx_sb = const.tile([128, ICT], _I16)
        nc.sync.dma_start(idx_sb[:], idx_d.ap())

        # get the gpsimd library load off the critical path: it can load
        # during the matmul instead of right before the first gather
        nc.gpsimd.load_library(library_config.ap_gather)

        TBLs = []
        # ---- per-partitioning pipeline: matmul -> softmax -> table ----
        # A (h=0) runs fully first so its gathers can start while B is
        # still in flight on the tensor/vector/scalar engines.
        for h in range(PPC):
            k0 = K * h
            ps = psum.tile([B, 2, 512], _F32, tag=f"ps{h}", name=f"ps{h}")
            wt = None
            for j in range(DCH):
                if j % WCH == 0:
                    wt = wpool.tile([128, WCH, K], _F8, tag="w")
                    off = (h * (DCH // WCH) + j // WCH) * (WCH * K)
                    nc.sync.dma_start(
                        wt[:],
                        w8_d[:, off : off + WCH * K].rearrange(
                            "p (c n) -> p c n", n=K
                        ),
                    )
                for n in range(2):
                    nc.tensor.matmul(
                        ps[:, n, 0:NT],
                        xt[:, j, :],
                        wt[:, j % WCH, NT * n : NT * (n + 1)],
                        start=(j == 0),
                        stop=False,
                    )
            for n in range(2):
                nc.tensor.matmul(
                    ps[:, n, 0:NT],
                    ones[:],
                    bias[:, k0 + NT * n : k0 + NT * (n + 1)],
                    start=False,
                    stop=True,
                )

            # softmax over the two PSUM banks of this partitioning
            mx = spool.tile([B, 2], _F32, tag=f"mx{h}")
            for n in range(2):
                nc.vector.reduce_max(
                    mx[:, n : n + 1], ps[:, n, 0:NT], axis=mybir.AxisListType.X
                )
            neg = spool.tile([B, 1], _F32, tag=f"neg{h}")
            nc.vector.tensor_tensor(
                neg[:], mx[:, 0:1], mx[:, 1:2], op=mybir.AluOpType.max
            )
            nc.vector.tensor_scalar_mul(neg[:], neg[:], -1.0)
            exp32 = spool.tile([B, K], _F32, tag=f"exp{h}")
            sacc = spool.tile([B, 2], _F32, tag=f"sacc{h}")
            for n in range(2):
                nc.scalar.activation(
                    exp32[:, NT * n : NT * (n + 1)],
                    ps[:, n, 0:NT],
                    mybir.ActivationFunctionType.Exp,
                    bias=neg[:],
                    accum_out=sacc[:, n : n + 1],
                )
            rec = spool.tile([B, 1], _F32, tag=f"rec{h}")
            nc.vector.tensor_tensor(
                rec[:], sacc[:, 0:1], sacc[:, 1:2], op=mybir.AluOpType.add
            )
            nc.vector.reciprocal(rec[:], rec[:])
            # scale probs x512 into fp8-e4m3's normal range (probs ~1e-3
            # sit at the subnormal floor otherwise); the uniform scale
            # cancels in the final class-normalization on the host
            nc.vector.tensor_scalar_mul(rec[:], rec[:], 512.0)
            probs16 = spool.tile([B, K], _F16, tag=f"p16{h}")
            nc.vector.tensor_scalar_mul(probs16[:], exp32[:], rec[:])

            # batch-interleaved gather table:
            # TBL[ch, m, j] = probs16[(ch % 16) + 16*j, m].
            # stage[ch, j, m] first: 4 interleave DMAs into group 0, then
            # 7 independent one-level replication DMAs (all contiguous;
            # a strided DMA would lower to a catastrophically slow
            # 2-byte-element transfer).  One strided vector copy then
            # does the (j, m) -> (m, j) interleave.
            stage = spool.tile([128, 4, K], _F16, tag=f"st{h}")
            for j in range(4):
                nc.sync.dma_start(
                    stage[0:16, j : j + 1, :],
                    probs16[16 * j : 16 * (j + 1), :].unsqueeze(1),
                )
            for q in range(1, NG):
                nc.sync.dma_start(
                    stage[16 * q : 16 * (q + 1), :, :], stage[0:16, :, :]
                )
            TBL = spool.tile([128, K, 4], _F8, tag=f"tbl{h}")
            nc.vector.tensor_copy(
                TBL[:, :, :].transpose([0, 2, 1]), stage[:, :, :]
            )
            TBLs.append(TBL)

        # ---- gather + store (gather layout; host sums and reorders) ----
        # chunk t: Q7 group q handles classes [CG*q + JCC*t, CG*q + JCC*(t+1))
        for t in range(NCH):
            for h in range(PPC):
                g = gpool.tile([128, NIDX, 4], _F8, tag="g")
                nc.gpsimd.ap_gather(
                    g[:, :, :],
                    TBLs[h][:, :, :],
                    idx_sb[:, ICOLS * (NCH * h + t) : ICOLS * (NCH * h + t + 1)],
                    channels=128,
                    num_elems=K,
                    d=4,
                    num_idxs=NIDX,
                )
                s = 2 * t + h
                oap = out_d.ap().rearrange(
                    "p (s a b) -> p s a b", s=2 * NCH, a=NIDX
                )
                if t == NCH - 1 and h == PPC - 1:
                    # last store is the exposed tail: halve it by running
                    # the two partition halves on separate queues (no
                    # gather left for the extra traffic to disturb)
                    nc.scalar.dma_start(oap[0:64, s, :, :], g[0:64, :, :])
                    nc.sync.dma_start(oap[64:128, s, :, :], g[64:128, :, :])
                else:
                    nc.scalar.dma_start(oap[:, s, :, :], g[:, :, :])

    nc.compile()
    return nc


def _host_inputs(x, W, b, part):
    """Per-core input maps: xT, wtb (W.T shard + bias row), gather indices."""
    import ml_dtypes

    # xT[p, (c, b)] = x[b, 128c + p]: contiguous per-partition DMA runs
    xT = np.ascontiguousarray(
        x.T.astype(ml_dtypes.float8_e4m3)
        .reshape(DCH, 128, B)
        .transpose(1, 0, 2)
        .reshape(128, DCH * B)
    )
    part = np.asarray(part).astype(np.int64, copy=False)
    in_maps = []
    for i in range(NCORES):
        r0 = NLOC * i
        # w8[p, (h, jb, c, n)] = W.T[128*(4jb + c) + p, 1000h + n]
        WT = W[r0 : r0 + NLOC].T.astype(ml_dtypes.float8_e4m3)
        w8 = np.ascontiguousarray(
            WT.reshape(DCH // WCH, WCH, 128, PPC, K)
            .transpose(2, 3, 0, 1, 4)
            .reshape(128, PPC * DCH * K)
        )
        bv = b[r0 : r0 + NLOC].astype(np.float16).reshape(1, NLOC)

        idxh = np.zeros((128, ICT), np.int16)
        for h in range(PPC):
            ph = (part[2 * i + h] - (2 * i + h) * K).astype(np.int16)  # [C] in [0, K)
            for q in range(NG):
                for t in range(NCH):
                    c0 = CG * q + JCC * t
                    s = np.zeros(ICOLS * 16, np.int16)
                    s[0:JCC] = ph[c0 : c0 + JCC]
                    blk = s.reshape(ICOLS, 16).T  # pos i <- (partition i%16, col i//16)
                    col = ICOLS * (NCH * h + t)
                    idxh[16 * q : 16 * (q + 1), col : col + ICOLS] = blk
        in_maps.append({"xT": xT, "w8": w8, "bv": bv, "idx": idxh})
    return in_maps


def kernel(**inputs):
    global LAST_RESULTS
    x = np.asarray(inputs["input"], dtype=np.float32)
    W = np.asarray(inputs["W"], dtype=np.float32)
    b = np.asarray(inputs["b"], dtype=np.float32)
    part = np.asarray(inputs["partitionings"])
    assert x.shape == (B, D) and W.shape == (P * K, D)

    if "nc" not in _CACHE:
        _CACHE["nc"] = _build_nc()
    nc = _CACHE["nc"]

    in_maps = _host_inputs(x, W, b, part)
    trace = bool(int(os.environ.get("BASSK_TRACE", "0")))
    res = bass_utils.run_bass_kernel_spmd(
        nc,
        in_maps,
        core_ids=list(range(NCORES)),
        trace=trace,
        tmpdir=os.environ.get("BASSK_TRACE_DIR") or None,
    )
    LAST_RESULTS = res

    # scratch layout: [128, t(NCH), h(PPC), NIDX, 4];
    # value at (ch, t, h, cc, j) = probs_{2*core+h}[b, part(...)] for
    # b = (ch%16) + 16*j, class c = CG*(ch//16) + JCC*t + cc (cc < JCC).
    acc = np.zeros((128, NCH, NIDX, 4), np.float32)
    for i in range(NCORES):
        arr = res.results[i]["part_out"].reshape(128, NCH, PPC, NIDX, 4)
        acc += arr[:, :, 0].astype(np.float32)
        acc += arr[:, :, 1].astype(np.float32)
    acc = acc[:, :, :JCC, :]                       # drop idx padding
    # (q, p', t, cc, j) -> out[b = p' + 16j, c = (q, t, cc)]
    out = np.ascontiguousarray(
        acc.reshape(NG, 16, NCH, JCC, 4).transpose(4, 1, 0, 2, 3)
    ).reshape(B, C)
    tot = out.sum(axis=1, keepdims=True)
    return np.log(out / tot + ESP).astype(np.float32)


# revision 26
# speedup vs baseline: 1.0123x; 1.0001x over previous
"""Trainium2 Bass kernel for nn_CombinatorialClassifier (segment_reduce).

Strategy (8 NeuronCores, tensor-parallel over the num_partitionings axis):
  - Core i owns partitionings {2i, 2i+1} (= A, B): a [2000, 2048] slice
    of W.
  - On device, per partitioning h in (A, B): logits_h = x @ W_h.T in
    fp8-e4m3 (+ fp16 b row folded into the matmul; quantization noise
    on the logits is ~0.05 abs, far inside the 2e-2 rel-err budget of
    the final log-probs), softmax -> probs_h [64, 1000] fp16,
    repacked into a batch-interleaved gather table
    TBL_h[ch, m, j] = probs_h[(ch%16) + 16*j, m] (d=4) replicated
    across all 8 Q7 core groups.  The A pipeline runs first so A
    gathers start while B is still in matmul/softmax.
  - Gather: one ap_gather index fetches the probs for FOUR batch rows
    at once, and each Q7 core group owns its own 6250-class range, so
    each Q7 core processes 2*6250 indices instead of 50000 -- ~3.3x
    less gpsimd command overhead than the d=1 layout.  Gather results
    are DMAed straight to a DRAM scratch in gather layout; the out-DMAs
    are issued from the scalar engine's queue so they cannot
    head-of-line-block the sync engine's serial DMA issue stream (which
    the B-table build needs), and no vector-engine work runs during the
    gathers (gpsimd shares its SBUF port with the vector engine).
  - Host: sum the 16 partials (8 cores x 2 partitionings) in the
    scratch layout, undo the batch interleave with one transpose, then
    normalize over classes and log.
"""

import os
from contextlib import ExitStack

import numpy as np

import concourse.bacc as bacc
import concourse.mybir as mybir
import concourse.tile as tile
from concourse import bass_utils
from concourse import library_config

B, P, K, C, D = 64, 16, 1000, 50000, 2048
ESP = 1e-20
NCORES = 8
PPC = P // NCORES        # partitionings per core (2)
NLOC = PPC * K           # local logits width (2000)
NT = 500                 # matmul N-tile (PSUM bank: 500 fp32 <= 512)
DCH = D // 128           # 16 contraction chunks of 128
WCH = 4                  # contraction chunks per W DMA

NG = 8                   # Q7 core groups (16 partitions each)
CG = C // NG             # classes per group (6250)
NCH = 5                  # gather chunks per group per partitioning
JCC = CG // NCH          # classes per chunk (1250)
NIDX = ((JCC + 3) // 4) * 4          # padded gather indices per call (1252)
ICOLS = ((NIDX + 31) // 32) * 2      # int16 idx cols per call (80)
ICT = 2 * NCH * ICOLS                # total idx cols (800)
SCR_W = 2 * NCH * NIDX * 4           # scratch free width (50080)

_F32 = mybir.dt.float32
_F8 = mybir.dt.float8e4
_F16 = mybir.dt.float16
_I16 = mybir.dt.int16

_CACHE = {}
LAST_RESULTS = None


def _build_nc():
    nc = bacc.Bacc(
        "TRN2",
        target_bir_lowering=False,
        debug=False,
        enable_asserts=False,
        num_devices=NCORES,
    )
    xT_d = nc.dram_tensor("xT", [128, DCH * B], _F8, kind="ExternalInput")
    w8_d = nc.dram_tensor("w8", [128, PPC * DCH * K], _F8, kind="ExternalInput")
    bv_d = nc.dram_tensor("bv", [1, NLOC], _F16, kind="ExternalInput")
    idx_d = nc.dram_tensor("idx", [128, ICT], _I16, kind="ExternalInput")
    out_d = nc.dram_tensor("part_out", [128, SCR_W], _F8, kind="ExternalOutput")

    with tile.TileContext(nc) as tc, ExitStack() as ctx:
        const = ctx.enter_context(tc.tile_pool(name="const", bufs=1))
        wpool = ctx.enter_context(tc.tile_pool(name="w", bufs=3))
        spool = ctx.enter_context(tc.tile_pool(name="stats", bufs=1))
        gpool = ctx.enter_context(tc.tile_pool(name="g", bufs=3))
        psum = ctx.enter_context(
            tc.tile_pool(name="psum", bufs=1, space="PSUM")
        )

        # host pre-permutes x and W so these DMAs are contiguous runs
        xt = const.tile([128, DCH, B], _F8)
        nc.sync.dma_start(xt[:], xT_d.ap().rearrange("p (c b) -> p c b", b=B))
        ones = const.tile([1, B], _F16)
        nc.vector.memset(ones[:], 1.0)
        bias = const.tile([1, NLOC], _F16)
        nc.sync.dma_start(bias[:], bv_d.ap())
        idx_sb = const.tile([128, ICT], _I16)
        nc.sync.dma_start(idx_sb[:], idx_d.ap())

        # get the gpsimd library load off the critical path: it can load
        # during the matmul instead of right before the first gather
        nc.gpsimd.load_library(library_config.ap_gather)

        TBLs = []
        # ---- per-partitioning pipeline: matmul -> softmax -> table ----
        # A (h=0) runs fully first so its gathers can start while B is
        # still in flight on the tensor/vector/scalar engines.
        for h in range(PPC):
            k0 = K * h
            ps = psum.tile([B, 2, 512], _F32, tag=f"ps{h}", name=f"ps{h}")
            wt = None
            for j in range(DCH):
                if j % WCH == 0:
                    wt = wpool.tile([128, WCH, K], _F8, tag="w")
                    off = (h * (DCH // WCH) + j // WCH) * (WCH * K)
                    nc.sync.dma_start(
                        wt[:],
                        w8_d[:, off : off + WCH * K].rearrange(
                            "p (c n) -> p c n", n=K
                        ),
                    )
                for n in range(2):
                    nc.tensor.matmul(
                        ps[:, n, 0:NT],
                        xt[:, j, :],
                        wt[:, j % WCH, NT * n : NT * (n + 1)],
                        start=(j == 0),
                        stop=False,
                    )
            for n in range(2):
                nc.tensor.matmul(
                    ps[:, n, 0:NT],
                    ones[:],
                    bias[:, k0 + NT * n : k0 + NT * (n + 1)],
                    start=False,
                    stop=True,
                )

            # softmax over the two PSUM banks of this partitioning
            mx = spool.tile([B, 2], _F32, tag=f"mx{h}")
            for n in range(2):
                nc.vector.reduce_max(
                    mx[:, n : n + 1], ps[:, n, 0:NT], axis=mybir.AxisListType.X
                )
            neg = spool.tile([B, 1], _F32, tag=f"neg{h}")
            nc.vector.tensor_tensor(
                neg[:], mx[:, 0:1], mx[:, 1:2], op=mybir.AluOpType.max
            )
            nc.vector.tensor_scalar_mul(neg[:], neg[:], -1.0)
            exp32 = spool.tile([B, K], _F32, tag=f"exp{h}")
            sacc = spool.tile([B, 2], _F32, tag=f"sacc{h}")
            for n in range(2):
                nc.scalar.activation(
                    exp32[:, NT * n : NT * (n + 1)],
                    ps[:, n, 0:NT],
                    mybir.ActivationFunctionType.Exp,
                    bias=neg[:],
                    accum_out=sacc[:, n : n + 1],
                )
            rec = spool.tile([B, 1], _F32, tag=f"rec{h}")
            nc.vector.tensor_tensor(
                rec[:], sacc[:, 0:1], sacc[:, 1:2], op=mybir.AluOpType.add
            )
            nc.vector.reciprocal(rec[:], rec[:])
            # scale probs x512 into fp8-e4m3's normal range (probs ~1e-3
            # sit at the subnormal floor otherwise); the uniform scale
            # cancels in the final class-normalization on the host
            nc.vector.tensor_scalar_mul(rec[:], rec[:], 512.0)
            probs16 = spool.tile([B, K], _F16, tag=f"p16{h}")
            nc.vector.tensor_scalar_mul(probs16[:], exp32[:], rec[:])

            # batch-interleaved gather table:
            # TBL[ch, m, j] = probs16[(ch % 16) + 16*j, m].
            # stage[ch, j, m] first: 4 interleave DMAs into group 0, then
            # 7 independent one-level replication DMAs (all contiguous;
            # a strided DMA would lower to a catastrophically slow
            # 2-byte-element transfer).  One strided vector copy then
            # does the (j, m) -> (m, j) interleave.
            stage = spool.tile([128, 4, K], _F16, tag=f"st{h}")
            for j in range(4):
                nc.sync.dma_start(
                    stage[0:16, j : j + 1, :],
                    probs16[16 * j : 16 * (j + 1), :].unsqueeze(1),
                )
            for q in range(1, NG):
                # spread A's replication copies across the three
                # DMA-capable queues -- they execute serially (~600ns
                # each) on the issuing engine, and only A's are on the
                # critical path to the first gather.  The gpsimd queue is
                # safe here: these run long before the first gather.
                if h == 0 and q >= 5:
                    eng = nc.gpsimd
                elif h == 0 and q >= 2:
                    eng = nc.scalar
                else:
                    eng = nc.sync
                eng.dma_start(
                    stage[16 * q : 16 * (q + 1), :, :], stage[0:16, :, :]
                )
            TBL = spool.tile([128, K, 4], _F8, tag=f"tbl{h}")
            nc.vector.tensor_copy(
                TBL[:, :, :].transpose([0, 2, 1]), stage[:, :, :]
            )
            TBLs.append(TBL)

        # ---- gather + store (gather layout; host sums and reorders) ----
        # chunk t: Q7 group q handles classes [CG*q + JCC*t, CG*q + JCC*(t+1))
        for t in range(NCH):
            for h in range(PPC):
                g = gpool.tile([128, NIDX, 4], _F8, tag="g")
                nc.gpsimd.ap_gather(
                    g[:, :, :],
                    TBLs[h][:, :, :],
                    idx_sb[:, ICOLS * (NCH * h + t) : ICOLS * (NCH * h + t + 1)],
                    channels=128,
                    num_elems=K,
                    d=4,
                    num_idxs=NIDX,
                )
                s = 2 * t + h
                oap = out_d.ap().rearrange(
                    "p (s a b) -> p s a b", s=2 * NCH, a=NIDX
                )
                if t == NCH - 1 and h == PPC - 1:
                    # last store is the exposed tail: halve it by running
                    # the two partition halves on separate queues (no
                    # gather left for the extra traffic to disturb)
                    nc.scalar.dma_start(oap[0:64, s, :, :], g[0:64, :, :])
                    nc.sync.dma_start(oap[64:128, s, :, :], g[64:128, :, :])
                else:
                    nc.scalar.dma_start(oap[:, s, :, :], g[:, :, :])

    nc.compile()
    return nc


def _host_inputs(x, W, b, part):
    """Per-core input maps: xT, wtb (W.T shard + bias row), gather indices."""
    import ml_dtypes

    # xT[p, (c, b)] = x[b, 128c + p]: contiguous per-partition DMA runs
    xT = np.ascontiguousarray(
        x.T.astype(ml_dtypes.float8_e4m3)
        .reshape(DCH, 128, B)
        .transpose(1, 0, 2)
        .reshape(128, DCH * B)
    )
    part = np.asarray(part).astype(np.int64, copy=False)
    in_maps = []
    for i in range(NCORES):
        r0 = NLOC * i
        # w8[p, (h, jb, c, n)] = W.T[128*(4jb + c) + p, 1000h + n]
        WT = W[r0 : r0 + NLOC].T.astype(ml_dtypes.float8_e4m3)
        w8 = np.ascontiguousarray(
            WT.reshape(DCH // WCH, WCH, 128, PPC, K)
            .transpose(2, 3, 0, 1, 4)
            .reshape(128, PPC * DCH * K)
        )
        bv = b[r0 : r0 + NLOC].astype(np.float16).reshape(1, NLOC)

        idxh = np.zeros((128, ICT), np.int16)
        for h in range(PPC):
            ph = (part[2 * i + h] - (2 * i + h) * K).astype(np.int16)  # [C] in [0, K)
            for q in range(NG):
                for t in range(NCH):
                    c0 = CG * q + JCC * t
                    s = np.zeros(ICOLS * 16, np.int16)
                    s[0:JCC] = ph[c0 : c0 + JCC]
                    blk = s.reshape(ICOLS, 16).T  # pos i <- (partition i%16, col i//16)
                    col = ICOLS * (NCH * h + t)
                    idxh[16 * q : 16 * (q + 1), col : col + ICOLS] = blk
        in_maps.append({"xT": xT, "w8": w8, "bv": bv, "idx": idxh})
    return in_maps


def kernel(**inputs):
    global LAST_RESULTS
    x = np.asarray(inputs["input"], dtype=np.float32)
    W = np.asarray(inputs["W"], dtype=np.float32)
    b = np.asarray(inputs["b"], dtype=np.float32)
    part = np.asarray(inputs["partitionings"])
    assert x.shape == (B, D) and W.shape == (P * K, D)

    if "nc" not in _CACHE:
        _CACHE["nc"] = _build_nc()
    nc = _CACHE["nc"]

    in_maps = _host_inputs(x, W, b, part)
    trace = bool(int(os.environ.get("BASSK_TRACE", "0")))
    res = bass_utils.run_bass_kernel_spmd(
        nc,
        in_maps,
        core_ids=list(range(NCORES)),
        trace=trace,
        tmpdir=os.environ.get("BASSK_TRACE_DIR") or None,
    )
    LAST_RESULTS = res

    # scratch layout: [128, t(NCH), h(PPC), NIDX, 4];
    # value at (ch, t, h, cc, j) = probs_{2*core+h}[b, part(...)] for
    # b = (ch%16) + 16*j, class c = CG*(ch//16) + JCC*t + cc (cc < JCC).
    acc = np.zeros((128, NCH, NIDX, 4), np.float32)
    for i in range(NCORES):
        arr = res.results[i]["part_out"].reshape(128, NCH, PPC, NIDX, 4)
        acc += arr[:, :, 0].astype(np.float32)
        acc += arr[:, :, 1].astype(np.float32)
    acc = acc[:, :, :JCC, :]                       # drop idx padding
    # (q, p', t, cc, j) -> out[b = p' + 16j, c = (q, t, cc)]
    out = np.ascontiguousarray(
        acc.reshape(NG, 16, NCH, JCC, 4).transpose(4, 1, 0, 2, 3)
    ).reshape(B, C)
    tot = out.sum(axis=1, keepdims=True)
    return np.log(out / tot + ESP).astype(np.float32)


# revision 27
# speedup vs baseline: 1.0138x; 1.0015x over previous
"""Trainium2 Bass kernel for nn_CombinatorialClassifier (segment_reduce).

Strategy (8 NeuronCores, tensor-parallel over the num_partitionings axis):
  - Core i owns partitionings {2i, 2i+1} (= A, B): a [2000, 2048] slice
    of W.
  - On device, per partitioning h in (A, B): logits_h = x @ W_h.T in
    fp8-e4m3 (+ fp16 b row folded into the matmul; quantization noise
    on the logits is ~0.05 abs, far inside the 2e-2 rel-err budget of
    the final log-probs), softmax -> probs_h [64, 1000] fp16,
    repacked into a batch-interleaved gather table
    TBL_h[ch, m, j] = probs_h[(ch%16) + 16*j, m] (d=4) replicated
    across all 8 Q7 core groups.  The A pipeline runs first so A
    gathers start while B is still in matmul/softmax.
  - Gather: one ap_gather index fetches the probs for FOUR batch rows
    at once, and each Q7 core group owns its own 6250-class range, so
    each Q7 core processes 2*6250 indices instead of 50000 -- ~3.3x
    less gpsimd command overhead than the d=1 layout.  Gather results
    are DMAed straight to a DRAM scratch in gather layout; the out-DMAs
    are issued from the scalar engine's queue so they cannot
    head-of-line-block the sync engine's serial DMA issue stream (which
    the B-table build needs), and no vector-engine work runs during the
    gathers (gpsimd shares its SBUF port with the vector engine).
  - Host: sum the 16 partials (8 cores x 2 partitionings) in the
    scratch layout, undo the batch interleave with one transpose, then
    normalize over classes and log.
"""

import os
from contextlib import ExitStack

import numpy as np

import concourse.bacc as bacc
import concourse.mybir as mybir
import concourse.tile as tile
from concourse import bass_utils
from concourse import library_config

B, P, K, C, D = 64, 16, 1000, 50000, 2048
ESP = 1e-20
NCORES = 8
PPC = P // NCORES        # partitionings per core (2)
NLOC = PPC * K           # local logits width (2000)
NT = 500                 # matmul N-tile (PSUM bank: 500 fp32 <= 512)
DCH = D // 128           # 16 contraction chunks of 128
WCH = 4                  # contraction chunks per W DMA

NG = 8                   # Q7 core groups (16 partitions each)
CG = C // NG             # classes per group (6250)
NCH = 5                  # gather chunks per group per partitioning
JCC = CG // NCH          # classes per chunk (1250)
NIDX = ((JCC + 3) // 4) * 4          # padded gather indices per call (1252)
ICOLS = ((NIDX + 31) // 32) * 2      # int16 idx cols per call (80)
ICT = 2 * NCH * ICOLS                # total idx cols (800)
SCR_W = 2 * NCH * NIDX * 4           # scratch free width (50080)

_F32 = mybir.dt.float32
_F8 = mybir.dt.float8e4
_F16 = mybir.dt.float16
_I16 = mybir.dt.int16

_CACHE = {}
LAST_RESULTS = None


def _build_nc():
    nc = bacc.Bacc(
        "TRN2",
        target_bir_lowering=False,
        debug=False,
        enable_asserts=False,
        num_devices=NCORES,
    )
    xT_d = nc.dram_tensor("xT", [128, DCH * B], _F8, kind="ExternalInput")
    w8_d = nc.dram_tensor("w8", [128, PPC * DCH * K], _F8, kind="ExternalInput")
    bv_d = nc.dram_tensor("bv", [1, NLOC], _F16, kind="ExternalInput")
    idx_d = nc.dram_tensor("idx", [128, ICT], _I16, kind="ExternalInput")
    msk_d = nc.dram_tensor("msk", [B, 128], _F16, kind="ExternalInput")
    out_d = nc.dram_tensor("part_out", [128, SCR_W], _F8, kind="ExternalOutput")

    with tile.TileContext(nc) as tc, ExitStack() as ctx:
        const = ctx.enter_context(tc.tile_pool(name="const", bufs=1))
        wpool = ctx.enter_context(tc.tile_pool(name="w", bufs=3))
        spool = ctx.enter_context(tc.tile_pool(name="stats", bufs=1))
        gpool = ctx.enter_context(tc.tile_pool(name="g", bufs=3))
        psum = ctx.enter_context(
            tc.tile_pool(name="psum", bufs=1, space="PSUM")
        )

        # host pre-permutes x and W so these DMAs are contiguous runs
        xt = const.tile([128, DCH, B], _F8)
        nc.sync.dma_start(xt[:], xT_d.ap().rearrange("p (c b) -> p c b", b=B))
        ones = const.tile([1, B], _F16)
        nc.vector.memset(ones[:], 1.0)
        bias = const.tile([1, NLOC], _F16)
        nc.sync.dma_start(bias[:], bv_d.ap())
        idx_sb = const.tile([128, ICT], _I16)
        nc.sync.dma_start(idx_sb[:], idx_d.ap())
        msk = const.tile([B, 128], _F16)
        nc.sync.dma_start(msk[:], msk_d.ap())

        # get the gpsimd library load off the critical path: it can load
        # during the matmul instead of right before the first gather
        nc.gpsimd.load_library(library_config.ap_gather)

        # R2 holds probs in diagonal j-major blocks (R2[b, b//16, :] =
        # probs[b, :]); the off-block zeros are set once here, off the
        # critical path, and survive since only diag blocks are rewritten
        R2s = []
        for h in range(PPC):
            R2 = spool.tile([B, 4, K], _F16, tag=f"R2{h}")
            nc.vector.memset(R2[:], 0.0)
            R2s.append(R2)

        TBLs = []
        # ---- per-partitioning pipeline: matmul -> softmax -> table ----
        # A (h=0) runs fully first so its gathers can start while B is
        # still in flight on the tensor/vector/scalar engines.
        for h in range(PPC):
            k0 = K * h
            ps = psum.tile([B, 2, 512], _F32, tag=f"ps{h}", name=f"ps{h}")
            wt = None
            for j in range(DCH):
                if j % WCH == 0:
                    wt = wpool.tile([128, WCH, K], _F8, tag="w")
                    off = (h * (DCH // WCH) + j // WCH) * (WCH * K)
                    nc.sync.dma_start(
                        wt[:],
                        w8_d[:, off : off + WCH * K].rearrange(
                            "p (c n) -> p c n", n=K
                        ),
                    )
                for n in range(2):
                    nc.tensor.matmul(
                        ps[:, n, 0:NT],
                        xt[:, j, :],
                        wt[:, j % WCH, NT * n : NT * (n + 1)],
                        start=(j == 0),
                        stop=False,
                    )
            for n in range(2):
                nc.tensor.matmul(
                    ps[:, n, 0:NT],
                    ones[:],
                    bias[:, k0 + NT * n : k0 + NT * (n + 1)],
                    start=False,
                    stop=True,
                )

            # softmax over the two PSUM banks of this partitioning
            mx = spool.tile([B, 2], _F32, tag=f"mx{h}")
            for n in range(2):
                nc.vector.reduce_max(
                    mx[:, n : n + 1], ps[:, n, 0:NT], axis=mybir.AxisListType.X
                )
            neg = spool.tile([B, 1], _F32, tag=f"neg{h}")
            nc.vector.tensor_tensor(
                neg[:], mx[:, 0:1], mx[:, 1:2], op=mybir.AluOpType.max
            )
            nc.vector.tensor_scalar_mul(neg[:], neg[:], -1.0)
            exp32 = spool.tile([B, K], _F32, tag=f"exp{h}")
            sacc = spool.tile([B, 2], _F32, tag=f"sacc{h}")
            for n in range(2):
                nc.scalar.activation(
                    exp32[:, NT * n : NT * (n + 1)],
                    ps[:, n, 0:NT],
                    mybir.ActivationFunctionType.Exp,
                    bias=neg[:],
                    accum_out=sacc[:, n : n + 1],
                )
            rec = spool.tile([B, 1], _F32, tag=f"rec{h}")
            nc.vector.tensor_tensor(
                rec[:], sacc[:, 0:1], sacc[:, 1:2], op=mybir.AluOpType.add
            )
            nc.vector.reciprocal(rec[:], rec[:])
            # scale probs x512 into fp8-e4m3's normal range (probs ~1e-3
            # sit at the subnormal floor otherwise); the uniform scale
            # cancels in the final class-normalization on the host
            nc.vector.tensor_scalar_mul(rec[:], rec[:], 512.0)
            probs16 = spool.tile([B, K], _F16, tag=f"p16{h}")
            nc.vector.tensor_scalar_mul(probs16[:], exp32[:], rec[:])

            # batch-interleaved gather table via mask matmul on the
            # tensor engine: TBL[ch, (m, j)] = probs[(ch%16)+16j, m]
            #   = sum_b msk[b, ch] * R2[b, b//16-block, m]
            # The j-minor output order comes from streaming R2 through a
            # transposed rhs view; PSUM -> SBUF fp8 quantize-copies run
            # on the scalar engine.  No DMA feeds the table, so the
            # first gather is not coupled to any DMA-semaphore rotation.
            R2 = R2s[h]
            for j in range(4):
                nc.sync.dma_start(
                    R2[16 * j : 16 * (j + 1), j : j + 1, :],
                    probs16[16 * j : 16 * (j + 1), :].unsqueeze(1),
                )
            R2v = R2[:, :, :].transpose([0, 2, 1])
            TBL = spool.tile([128, K, 4], _F8, tag=f"tbl{h}")
            ptbl = psum.tile([128, 2, 512], _F32, tag=f"pt{h}", name=f"pt{h}")
            MT = K // 8
            for c in range(8):
                nc.tensor.matmul(
                    ptbl[:, c % 2, 0 : 4 * MT],
                    msk[:],
                    R2v[:, MT * c : MT * (c + 1), :],
                    start=True,
                    stop=True,
                )
                nc.scalar.copy(
                    TBL[:, MT * c : MT * (c + 1), :].rearrange("p a b -> p (a b)"),
                    ptbl[:, c % 2, 0 : 4 * MT],
                )
            TBLs.append(TBL)

        # ---- gather + store (gather layout; host sums and reorders) ----
        # chunk t: Q7 group q handles classes [CG*q + JCC*t, CG*q + JCC*(t+1))
        for t in range(NCH):
            for h in range(PPC):
                g = gpool.tile([128, NIDX, 4], _F8, tag="g")
                nc.gpsimd.ap_gather(
                    g[:, :, :],
                    TBLs[h][:, :, :],
                    idx_sb[:, ICOLS * (NCH * h + t) : ICOLS * (NCH * h + t + 1)],
                    channels=128,
                    num_elems=K,
                    d=4,
                    num_idxs=NIDX,
                )
                s = 2 * t + h
                oap = out_d.ap().rearrange(
                    "p (s a b) -> p s a b", s=2 * NCH, a=NIDX
                )
                if t == NCH - 1 and h == PPC - 1:
                    # last store is the exposed tail: halve it by running
                    # the two partition halves on separate queues (no
                    # gather left for the extra traffic to disturb)
                    nc.scalar.dma_start(oap[0:64, s, :, :], g[0:64, :, :])
                    nc.sync.dma_start(oap[64:128, s, :, :], g[64:128, :, :])
                else:
                    nc.scalar.dma_start(oap[:, s, :, :], g[:, :, :])

    nc.compile()
    return nc


def _host_inputs(x, W, b, part):
    """Per-core input maps: xT, wtb (W.T shard + bias row), gather indices."""
    import ml_dtypes

    # xT[p, (c, b)] = x[b, 128c + p]: contiguous per-partition DMA runs
    xT = np.ascontiguousarray(
        x.T.astype(ml_dtypes.float8_e4m3)
        .reshape(DCH, 128, B)
        .transpose(1, 0, 2)
        .reshape(128, DCH * B)
    )
    part = np.asarray(part).astype(np.int64, copy=False)
    bvec = np.arange(B)[:, None]
    cvec = np.arange(128)[None, :]
    msk = ((bvec % 16) == (cvec % 16)).astype(np.float16)
    in_maps = []
    for i in range(NCORES):
        r0 = NLOC * i
        # w8[p, (h, jb, c, n)] = W.T[128*(4jb + c) + p, 1000h + n]
        WT = W[r0 : r0 + NLOC].T.astype(ml_dtypes.float8_e4m3)
        w8 = np.ascontiguousarray(
            WT.reshape(DCH // WCH, WCH, 128, PPC, K)
            .transpose(2, 3, 0, 1, 4)
            .reshape(128, PPC * DCH * K)
        )
        bv = b[r0 : r0 + NLOC].astype(np.float16).reshape(1, NLOC)

        idxh = np.zeros((128, ICT), np.int16)
        for h in range(PPC):
            ph = (part[2 * i + h] - (2 * i + h) * K).astype(np.int16)  # [C] in [0, K)
            for q in range(NG):
                for t in range(NCH):
                    c0 = CG * q + JCC * t
                    s = np.zeros(ICOLS * 16, np.int16)
                    s[0:JCC] = ph[c0 : c0 + JCC]
                    blk = s.reshape(ICOLS, 16).T  # pos i <- (partition i%16, col i//16)
                    col = ICOLS * (NCH * h + t)
                    idxh[16 * q : 16 * (q + 1), col : col + ICOLS] = blk
        in_maps.append({"xT": xT, "w8": w8, "bv": bv, "idx": idxh, "msk": msk})
    return in_maps


def kernel(**inputs):
    global LAST_RESULTS
    x = np.asarray(inputs["input"], dtype=np.float32)
    W = np.asarray(inputs["W"], dtype=np.float32)
    b = np.asarray(inputs["b"], dtype=np.float32)
    part = np.asarray(inputs["partitionings"])
    assert x.shape == (B, D) and W.shape == (P * K, D)

    if "nc" not in _CACHE:
        _CACHE["nc"] = _build_nc()
    nc = _CACHE["nc"]

    in_maps = _host_inputs(x, W, b, part)
    trace = bool(int(os.environ.get("BASSK_TRACE", "0")))
    res = bass_utils.run_bass_kernel_spmd(
        nc,
        in_maps,
        core_ids=list(range(NCORES)),
        trace=trace,
        tmpdir=os.environ.get("BASSK_TRACE_DIR") or None,
    )
    LAST_RESULTS = res

    # scratch layout: [128, t(NCH), h(PPC), NIDX, 4];
    # value at (ch, t, h, cc, j) = probs_{2*core+h}[b, part(...)] for
    # b = (ch%16) + 16*j, class c = CG*(ch//16) + JCC*t + cc (cc < JCC).
    acc = np.zeros((128, NCH, NIDX, 4), np.float32)
    for i in range(NCORES):
        arr = res.results[i]["part_out"].reshape(128, NCH, PPC, NIDX, 4)
        acc += arr[:, :, 0].astype(np.float32)
        acc += arr[:, :, 1].astype(np.float32)
    acc = acc[:, :, :JCC, :]                       # drop idx padding
    # (q, p', t, cc, j) -> out[b = p' + 16j, c = (q, t, cc)]
    out = np.ascontiguousarray(
        acc.reshape(NG, 16, NCH, JCC, 4).transpose(4, 1, 0, 2, 3)
    ).reshape(B, C)
    tot = out.sum(axis=1, keepdims=True)
    return np.log(out / tot + ESP).astype(np.float32)
